# revision 7
# baseline (speedup 1.0000x reference)
"""Trainium2 Bass kernel: causal multi-head attention (dense transformer block).

Reference semantics (note the quirk: scores = K @ Q^T, scaled by C**-0.5):
    k = x @ Wk ; q = x @ Wq ; v = x @ Wv          (per-head split, H=16, D=64)
    wei[i, j] = (k_i . q_j) * C**-0.5,  masked j <= i, softmax over j
    out = (wei @ v) @ Wproj + bproj

Sharding (8 cores): data-parallel over batch (4) x tensor-parallel over
head-halves (2).  Core c handles batch c//2, heads [8*(c%2), 8*(c%2)+8).
Each core returns the transposed partial projection output
outT = ((attn_heads @ Wproj_shard) + bproj/2)^T ; the host sums the two
partials per batch and transposes back.

Device kernel layout choices:
  - "row" operand Qr := x @ Wk_shard, "col" operand Kc := x @ Wq_shard.
  - Qr^T / Kc^T are kept head-transposed in SBUF as [128 (= 2 heads x 64d), T].
  - scores are computed transposed: S^T[j, i] = Qr_i . Kc_j via
    lhsT = KcT (stationary), rhs = QrT (moving); two heads packed in the PE
    array via row groups (K=64 each, tile_position (0,0)/(64,0)).
  - softmax: exp on ACT (scale fused), causal mask via gpsimd.affine_select.
    The denominator is folded into the AV matmul: V is augmented with a ones
    column (M=65), so PSUM row 64 of the AV output accumulates sum_j wei.
    Reciprocal = exp(-ln(d)) (exp and ln share one ACT table set), broadcast
    across partitions via a DRAM-bounce DMA with a partition-broadcast AP.
  - AV: lhsT = V tiles (natural layout), rhs = exp'd S^T -> attn^T, which is
    exactly the layout the output projection wants as its moving operand.
  - all matmuls run as float32r (fp32 data, FP22 multiply): full PE rate for
    moving dim >= 256.
"""

import numpy as np

import concourse.bass as bass  # noqa: F401
import concourse.tile as tile
from concourse import bacc, mybir
from concourse.bass_utils import run_bass_kernel_spmd
from concourse.masks import make_identity

F32 = mybir.dt.float32
F32R = mybir.dt.float32r

B, T_FULL, C = 4, 2048, 1024
H, NCORES, P, NS, D = 16, 8, 128, 512, 64
CH = C // 2            # per-core channels (8 heads)
HPC = CH // D          # heads per core
SCALE = 1.0 / float(np.sqrt(C))


def _emit(ctx, tc, aps, T):
    nc = tc.nc
    x, wr, wc, wv, wp, bias_d, outT = (
        aps[k] for k in ("x", "wr", "wc", "wv", "wp", "bias", "outT")
    )
    KT = C // P           # 8 k-tiles of the model dim
    TT = T // P           # t-tiles
    TS = T // NS          # i-spans
    MT = CH // P          # 4 head-pair tiles
    CHUNK = NS
    NCH = T // CHUNK
    Exp = mybir.ActivationFunctionType.Exp
    Ln = mybir.ActivationFunctionType.Ln
    Copy = mybir.ActivationFunctionType.Copy

    consts = ctx.enter_context(tc.tile_pool(name="consts", bufs=1))
    ident = consts.tile([P, P], F32)
    make_identity(nc, ident)
    onesf = consts.tile([P, P], F32)
    nc.vector.memset(onesf, 1.0)
    bias_sb = consts.tile([P, C // P], F32)
    nc.sync.dma_start(out=bias_sb, in_=bias_d)

    acts = ctx.enter_context(tc.tile_pool(name="acts", bufs=1))
    qrT = acts.tile([P, MT, T], F32R)      # row operand, head-transposed
    kcT = acts.tile([P, MT, T], F32R)      # col operand, head-transposed
    v_sb = acts.tile([P, TT, HPC, D + 1], F32R)  # V + ones column per head
    # ones column (col 64 of every head slot), written once
    nc.vector.tensor_copy(
        v_sb[:, :, :, D : D + 1],
        onesf[:, 0 : TT * HPC].rearrange("p (a b c) -> p a b c", a=TT, b=HPC, c=1),
    )

    # ---------------- phase 1: projections ----------------
    with (
        tc.tile_pool(name="wqkv", bufs=1) as wpool,
        tc.tile_pool(name="xn", bufs=2) as xnp,
        tc.tile_pool(name="xt", bufs=2) as xtp,
        tc.tile_pool(name="pt", bufs=2, space="PSUM") as ptp,
        tc.tile_pool(name="pqv", bufs=3, space="PSUM") as pqp,
    ):
        wr_sb = wpool.tile([P, KT, CH], F32R)
        wc_sb = wpool.tile([P, KT, CH], F32R)
        wv_sb = wpool.tile([P, KT, CH], F32R)
        for w_dram, w_sb in ((wr, wr_sb), (wc, wc_sb), (wv, wv_sb)):
            nc.sync.dma_start(
                out=w_sb,
                in_=w_dram.bitcast(F32R).rearrange("(kt p) ch -> p kt ch", p=P),
            )
        for chk in range(NCH):
            t0 = chk * CHUNK
            ntl = CHUNK // P
            xt = xtp.tile([P, KT, CHUNK], F32R)
            for tl in range(ntl):
                xn = xnp.tile([P, C], F32)
                nc.sync.dma_start(out=xn, in_=x[t0 + tl * P : t0 + (tl + 1) * P, :])
                for kt in range(KT):
                    pt = ptp.tile([P, P], F32)
                    nc.tensor.transpose(pt, xn[:, kt * P : (kt + 1) * P], ident)
                    nc.vector.tensor_copy(xt[:, kt, tl * P : (tl + 1) * P], pt)
            for w_sb, dst in ((wr_sb, qrT), (wc_sb, kcT)):
                for m in range(MT):
                    pq = pqp.tile([P, CHUNK], F32, tag="pqv")
                    for kt in range(KT):
                        nc.tensor.matmul(
                            pq,
                            w_sb[:, kt, m * P : (m + 1) * P],
                            xt[:, kt, :],
                            start=(kt == 0),
                            stop=(kt == KT - 1),
                        )
                    nc.scalar.activation(dst[:, m, t0 : t0 + CHUNK], pq, Copy)
            for tl in range(ntl):
                pv = pqp.tile([P, CH], F32, tag="pqv")
                for kt in range(KT):
                    nc.tensor.matmul(
                        pv,
                        xt[:, kt, tl * P : (tl + 1) * P],
                        wv_sb[:, kt, :],
                        start=(kt == 0),
                        stop=(kt == KT - 1),
                    )
                nc.scalar.activation(
                    v_sb[:, chk * ntl + tl, :, 0:D],
                    pv.rearrange("p (h d) -> p h d", h=HPC),
                    Copy,
                )

    # ---------------- phase 2: attention ----------------
    attn_pool = ctx.enter_context(tc.tile_pool(name="attn", bufs=1))
    attnT = attn_pool.tile([P, MT, T], F32R)
    with (
        tc.tile_pool(name="wei", bufs=3) as weip,
        tc.tile_pool(name="lnr", bufs=2) as lnrp,
        tc.tile_pool(name="bcs", bufs=2) as bcsp,
        tc.tile_pool(name="rdram", bufs=2, space="DRAM") as rdp,
        tc.tile_pool(name="psc", bufs=2, space="PSUM") as pscp,
        tc.tile_pool(name="pava", bufs=2, space="PSUM") as pavap,
        tc.tile_pool(name="pavb", bufs=2, space="PSUM") as pavbp,
    ):
        for p in range(MT):
            for s in range(TS):
                i0 = s * NS
                jmax = (s + 1) * (NS // P)
                pav = [
                    pavap.tile([D + 1, NS], F32, tag="pava", name="pava"),
                    pavbp.tile([D + 1, NS], F32, tag="pavb", name="pavb"),
                ]
                for jt in range(jmax):
                    off = max(0, jt * P - i0)
                    first, last = jt == 0, jt == jmax - 1
                    psc = pscp.tile([P, 2, NS], F32, tag="psc")
                    for h in range(2):
                        hp = slice(64 * h, 64 * h + 64)
                        nc.tensor.matmul(
                            psc[:, h, off:NS],
                            kcT[hp, p, jt * P : (jt + 1) * P],
                            qrT[hp, p, i0 + off : i0 + NS],
                            start=True,
                            stop=True,
                            tile_position=(64 * h, 0),
                        )
                    wei = weip.tile([P, 2, NS], F32R, tag="wei")
                    nc.scalar.activation(
                        wei[:, :, off:NS], psc[:, :, off:NS], Exp, scale=SCALE
                    )
                    if jt * P >= i0:  # tile containing the diagonal
                        nc.gpsimd.affine_select(
                            out=wei[:, :, off : off + P],
                            in_=wei[:, :, off : off + P],
                            pattern=[[0, 2], [1, P]],
                            base=0,
                            channel_multiplier=-1,
                            compare_op=mybir.AluOpType.is_ge,
                            fill=0.0,
                        )
                    for h in range(2):
                        nc.tensor.matmul(
                            pav[h][:, off:NS],
                            v_sb[:, jt, 2 * p + h, :],
                            wei[:, h, off:NS],
                            start=first,
                            stop=last,
                        )
                # normalize: attnT[:, p, span] = pav[0:64] * 1/pav[64]
                lr = lnrp.tile([P, 2, NS], F32, tag="lr")
                rab = lnrp.tile([P, 2, NS], F32, tag="rab")
                for h in range(2):
                    nc.scalar.activation(lr[D : D + 1, h, :], pav[h][D : D + 1, :], Ln)
                    nc.scalar.activation(
                        rab[D : D + 1, h, :], lr[D : D + 1, h, :], Exp, scale=-1.0
                    )
                rd = rdp.tile([1, 2, NS], F32, tag="rd")
                nc.gpsimd.dma_start(out=rd[0], in_=rab[D : D + 1, :, :])
                bcs = bcsp.tile([D, 2, NS], F32, tag="bcs")
                nc.gpsimd.dma_start(out=bcs, in_=rd.to_broadcast([D, 2, NS]))
                for h in range(2):
                    nc.vector.tensor_mul(
                        attnT[64 * h : 64 * h + 64, p, i0 : i0 + NS],
                        pav[h][0:D, :],
                        bcs[:, h, :],
                    )

    # ---------------- phase 3: output projection ----------------
    with (
        tc.tile_pool(name="wpp", bufs=1) as wpp,
        tc.tile_pool(name="po_sb", bufs=3) as posp,
        tc.tile_pool(name="po", bufs=3, space="PSUM") as pop,
    ):
        wp_sb = wpp.tile([P, MT, C], F32R)
        nc.sync.dma_start(
            out=wp_sb, in_=wp.bitcast(F32R).rearrange("(cp p) c -> p cp c", p=P)
        )
        for m in range(C // P):
            for s in range(TS):
                po = pop.tile([P, NS], F32, tag="po")
                for cp in range(MT):
                    nc.tensor.matmul(
                        po,
                        wp_sb[:, cp, m * P : (m + 1) * P],
                        attnT[:, cp, s * NS : (s + 1) * NS],
                        start=(cp == 0),
                        stop=(cp == MT - 1),
                    )
                po_sb = posp.tile([P, NS], F32, tag="po_sb")
                nc.vector.tensor_scalar_add(po_sb, po, bias_sb[:, m : m + 1])
                nc.sync.dma_start(
                    out=outT[m * P : (m + 1) * P, s * NS : (s + 1) * NS], in_=po_sb
                )


def build(T=T_FULL, reps=1):
    from contextlib import ExitStack

    nc = bacc.Bacc(
        "TRN2", target_bir_lowering=False, debug=False, num_devices=NCORES
    )
    aps = {
        "x": nc.dram_tensor("x", [T, C], F32, kind="ExternalInput").ap(),
        "wr": nc.dram_tensor("wr", [C, CH], F32, kind="ExternalInput").ap(),
        "wc": nc.dram_tensor("wc", [C, CH], F32, kind="ExternalInput").ap(),
        "wv": nc.dram_tensor("wv", [C, CH], F32, kind="ExternalInput").ap(),
        "wp": nc.dram_tensor("wp", [CH, C], F32, kind="ExternalInput").ap(),
        "bias": nc.dram_tensor("bias", [P, C // P], F32, kind="ExternalInput").ap(),
        "outT": nc.dram_tensor("outT", [C, T], F32, kind="ExternalOutput").ap(),
    }
    with tile.TileContext(nc) as tc:
        with ExitStack() as ctx:
            if reps == 1:
                _emit(ctx, tc, aps, T)
            else:
                with tc.For_i(0, reps, 1):
                    _emit(ctx, tc, aps, T)
    nc.compile()
    return nc


def make_in_maps(x, Wk, Wq, Wv, Wproj, bproj):
    """Shard full inputs into 8 per-core input maps."""
    in_maps = []
    for c in range(NCORES):
        b, g = c // 2, c % 2
        cols = slice(CH * g, CH * (g + 1))
        in_maps.append(
            {
                "x": np.ascontiguousarray(np.asarray(x)[b], dtype=np.float32),
                "wr": np.ascontiguousarray(np.asarray(Wk)[:, cols], dtype=np.float32),
                "wc": np.ascontiguousarray(np.asarray(Wq)[:, cols], dtype=np.float32),
                "wv": np.ascontiguousarray(np.asarray(Wv)[:, cols], dtype=np.float32),
                "wp": np.ascontiguousarray(np.asarray(Wproj)[cols, :], dtype=np.float32),
                "bias": np.ascontiguousarray(
                    (0.5 * np.asarray(bproj)).reshape(C // P, P).T, dtype=np.float32
                ),
            }
        )
    return in_maps


_CACHE = {}


def kernel(x, Wk, Wq, Wv, Wproj, bproj):
    x = np.asarray(x, dtype=np.float32)
    if "nc" not in _CACHE:
        _CACHE["nc"] = build(T=x.shape[1])
    nc = _CACHE["nc"]
    in_maps = make_in_maps(x, Wk, Wq, Wv, Wproj, bproj)
    res = run_bass_kernel_spmd(nc, in_maps, list(range(NCORES)))
    out = np.empty((x.shape[0], x.shape[1], C), dtype=np.float32)
    for b in range(x.shape[0]):
        out[b] = (res.results[2 * b]["outT"] + res.results[2 * b + 1]["outT"]).T
    return out


# revision 14
# speedup vs baseline: 64.7269x; 64.7269x over previous
"""Trainium2 Bass kernel: causal multi-head attention (dense transformer block).

Reference semantics (note the quirk: scores = K @ Q^T, scaled by C**-0.5):
    k = x @ Wk ; q = x @ Wq ; v = x @ Wv          (per-head split, H=16, D=64)
    wei[i, j] = (k_i . q_j) * C**-0.5,  masked j <= i, softmax over j
    out = (wei @ v) @ Wproj + bproj

Sharding (8 cores): data-parallel over batch (4) x tensor-parallel over
head-halves (2).  Core c handles batch c//2, heads [8*(c%2), 8*(c%2)+8).
Each core returns the transposed partial projection output
outT = ((attn_heads @ Wproj_shard) + bproj/2)^T ; the host sums the two
partials per batch and transposes back.

Device kernel layout choices:
  - "row" operand Qr := x @ Wk_shard, "col" operand Kc := x @ Wq_shard.
  - Qr^T / Kc^T are kept head-transposed in SBUF as [128 (= 2 heads x 64d), T].
  - scores are computed transposed: S^T[j, i] = Qr_i . Kc_j via
    lhsT = KcT (stationary), rhs = QrT (moving); two heads packed in the PE
    array via row groups (K=64 each, tile_position (0,0)/(64,0)).
  - softmax: exp on ACT (scale fused), causal mask via gpsimd.affine_select.
    The denominator is folded into the AV matmul: V is augmented with a ones
    column (M=65), so PSUM row 64 of the AV output accumulates sum_j wei.
    Reciprocal = exp(-ln(d)) (exp and ln share one ACT table set), broadcast
    across partitions via a DRAM-bounce DMA with a partition-broadcast AP.
  - AV: lhsT = V tiles (natural layout), rhs = exp'd S^T -> attn^T, which is
    exactly the layout the output projection wants as its moving operand.
  - all matmuls run as float32r (fp32 data, FP22 multiply): full PE rate for
    moving dim >= 256.
"""

import numpy as np

import concourse.bass as bass  # noqa: F401
import concourse.tile as tile
from concourse import bacc, mybir
from concourse.bass_utils import run_bass_kernel_spmd
from concourse.masks import make_identity

F32 = mybir.dt.float32
F32R = mybir.dt.float32r

B, T_FULL, C = 4, 2048, 1024
H, NCORES, P, NS, D = 16, 8, 128, 512, 64
CH = C // 2            # per-core channels (8 heads)
HPC = CH // D          # heads per core
SCALE = 1.0 / float(np.sqrt(C))


def _emit(ctx, tc, aps, T, phases=(1, 2, 3)):
    nc = tc.nc
    x, wr, wc, wv, wp, bias_d, outT = (
        aps[k] for k in ("x", "wr", "wc", "wv", "wp", "bias", "outT")
    )
    KT = C // P           # 8 k-tiles of the model dim
    TT = T // P           # t-tiles
    TS = T // NS          # i-spans
    MT = CH // P          # 4 head-pair tiles
    CHUNK = NS
    NCH = T // CHUNK
    Exp = mybir.ActivationFunctionType.Exp
    Ln = mybir.ActivationFunctionType.Ln
    Copy = mybir.ActivationFunctionType.Copy

    consts = ctx.enter_context(tc.tile_pool(name="consts", bufs=1))
    ident = consts.tile([P, P], F32)
    make_identity(nc, ident)
    onesf = consts.tile([P, P], F32)
    nc.vector.memset(onesf, 1.0)
    bias_sb = consts.tile([P, C // P], F32)
    nc.sync.dma_start(out=bias_sb, in_=bias_d)

    acts = ctx.enter_context(tc.tile_pool(name="acts", bufs=1))
    qrT = acts.tile([P, MT, T], F32R)      # row operand, head-transposed
    kcT = acts.tile([P, MT, T], F32R)      # col operand, head-transposed
    v_sb = acts.tile([P, TT, HPC, D + 1], F32R)  # V + ones column per head
    # ones column (col 64 of every head slot), written once
    nc.vector.tensor_copy(
        v_sb[:, :, :, D : D + 1],
        onesf[:, 0 : TT * HPC].rearrange("p (a b c) -> p a b c", a=TT, b=HPC, c=1),
    )

    # ---------------- phase 1: projections ----------------
    if 1 not in phases:
        return
    with (
        tc.tile_pool(name="wqkv", bufs=1) as wpool,
        tc.tile_pool(name="xn", bufs=2) as xnp,
        tc.tile_pool(name="xt", bufs=2) as xtp,
        tc.tile_pool(name="pt", bufs=2, space="PSUM") as ptp,
        tc.tile_pool(name="pqv", bufs=3, space="PSUM") as pqp,
    ):
        wr_sb = wpool.tile([P, KT, CH], F32R)
        wc_sb = wpool.tile([P, KT, CH], F32R)
        wv_sb = wpool.tile([P, KT, CH], F32R)
        for w_dram, w_sb in ((wr, wr_sb), (wc, wc_sb), (wv, wv_sb)):
            nc.gpsimd.dma_start(
                out=w_sb,
                in_=w_dram.bitcast(F32R).rearrange("(kt p) ch -> p kt ch", p=P),
            )
        for chk in range(NCH):
            t0 = chk * CHUNK
            ntl = CHUNK // P
            xt = xtp.tile([P, KT, CHUNK], F32R)
            for tl in range(ntl):
                xn = xnp.tile([P, C], F32)
                nc.sync.dma_start(out=xn, in_=x[t0 + tl * P : t0 + (tl + 1) * P, :])
                for kt in range(KT):
                    pt = ptp.tile([P, P], F32)
                    nc.tensor.transpose(pt, xn[:, kt * P : (kt + 1) * P], ident)
                    nc.vector.tensor_copy(xt[:, kt, tl * P : (tl + 1) * P], pt)
            for w_sb, dst in ((wr_sb, qrT), (wc_sb, kcT)):
                for m in range(MT):
                    pq = pqp.tile([P, CHUNK], F32, tag="pqv")
                    for kt in range(KT):
                        nc.tensor.matmul(
                            pq,
                            w_sb[:, kt, m * P : (m + 1) * P],
                            xt[:, kt, :],
                            start=(kt == 0),
                            stop=(kt == KT - 1),
                        )
                    nc.scalar.activation(dst[:, m, t0 : t0 + CHUNK], pq, Copy)
            for tl in range(ntl):
                pv = pqp.tile([P, CH], F32, tag="pqv")
                for kt in range(KT):
                    nc.tensor.matmul(
                        pv,
                        xt[:, kt, tl * P : (tl + 1) * P],
                        wv_sb[:, kt, :],
                        start=(kt == 0),
                        stop=(kt == KT - 1),
                    )
                nc.scalar.activation(
                    v_sb[:, chk * ntl + tl, :, 0:D],
                    pv.rearrange("p (h d) -> p h d", h=HPC),
                    Copy,
                )

    # ------- phases 2+3: attention with interleaved output projection -------
    attn_pool = ctx.enter_context(tc.tile_pool(name="attn", bufs=1))
    attnT = attn_pool.tile([P, MT, T], F32R)
    if 2 not in phases:
        return
    with (
        tc.tile_pool(name="wei", bufs=4) as weip,
        tc.tile_pool(name="lnr", bufs=2) as lnrp,
        tc.tile_pool(name="bcs", bufs=2) as bcsp,
        tc.tile_pool(name="rdram", bufs=2, space="DRAM") as rdp,
        tc.tile_pool(name="psc", bufs=2, space="PSUM") as pscp,
        tc.tile_pool(name="pava", bufs=2, space="PSUM") as pavap,
        tc.tile_pool(name="pavb", bufs=2, space="PSUM") as pavbp,
    ):
        units = [
            (p, s, jt)
            for p in range(MT)
            for s in range(TS)
            for jt in range((s + 1) * (NS // P))
        ]
        DEPTH = 2
        pend = {}
        pavs = {}

        def emit_front(u):
            p, s, jt = u
            i0 = s * NS
            off = max(0, jt * P - i0)
            psc = pscp.tile([P, 2, NS], F32, tag="psc", name="psc")
            for h in range(2):
                hp = slice(64 * h, 64 * h + 64)
                nc.tensor.matmul(
                    psc[:, h, off:NS],
                    kcT[hp, p, jt * P : (jt + 1) * P],
                    qrT[hp, p, i0 + off : i0 + NS],
                    start=True,
                    stop=True,
                    tile_position=(64 * h, 0),
                )
            wei = weip.tile([P, 2, NS], F32R, tag="wei", name="wei")
            nc.scalar.activation(
                wei[:, :, off:NS], psc[:, :, off:NS], Exp, scale=SCALE
            )
            if jt * P >= i0:  # tile containing the diagonal
                nc.gpsimd.affine_select(
                    out=wei[:, :, off : off + P],
                    in_=wei[:, :, off : off + P],
                    pattern=[[0, 2], [1, P]],
                    base=0,
                    channel_multiplier=-1,
                    compare_op=mybir.AluOpType.is_ge,
                    fill=0.0,
                )
            pend[u] = wei

        def emit_back(u):
            p, s, jt = u
            i0 = s * NS
            jmax = (s + 1) * (NS // P)
            off = max(0, jt * P - i0)
            first, last = jt == 0, jt == jmax - 1
            wei = pend.pop(u)
            if first:
                pavs[(p, s)] = [
                    pavap.tile([D + 1, NS], F32, tag="pava", name="pava"),
                    pavbp.tile([D + 1, NS], F32, tag="pavb", name="pavb"),
                ]
            pav = pavs[(p, s)]
            for h in range(2):
                nc.tensor.matmul(
                    pav[h][:, off:NS],
                    v_sb[:, jt, 2 * p + h, :],
                    wei[:, h, off:NS],
                    start=first,
                    stop=last,
                )
            if not last:
                return
            # normalize: attnT[:, p, span] = pav[0:64] * 1/pav[64]
            pav = pavs.pop((p, s))
            lr = lnrp.tile([P, 2, NS], F32, tag="lr", name="lr")
            rab = lnrp.tile([P, 2, NS], F32, tag="rab", name="rab")
            for h in range(2):
                nc.scalar.activation(lr[D : D + 1, h, :], pav[h][D : D + 1, :], Ln)
                nc.scalar.activation(
                    rab[D : D + 1, h, :], lr[D : D + 1, h, :], Exp, scale=-1.0
                )
            rd = rdp.tile([1, 2, NS], F32, tag="rd", name="rd")
            nc.gpsimd.dma_start(out=rd[0], in_=rab[D : D + 1, :, :])
            bcs = bcsp.tile([D, 2, NS], F32, tag="bcs", name="bcs")
            nc.gpsimd.dma_start(out=bcs, in_=rd.to_broadcast([D, 2, NS]))
            for h in range(2):
                nc.vector.tensor_mul(
                    attnT[64 * h : 64 * h + 64, p, s * NS : (s + 1) * NS],
                    pav[h][0:D, :],
                    bcs[:, h, :],
                )

        for idx in range(len(units) + DEPTH):
            if idx < len(units):
                emit_front(units[idx])
            if idx >= DEPTH:
                emit_back(units[idx - DEPTH])

    # ---------------- phase 3: output projection ----------------
    if 3 not in phases:
        return
    with (
        tc.tile_pool(name="wpp", bufs=1) as wpp,
        tc.tile_pool(name="po_sb", bufs=3) as posp,
        tc.tile_pool(name="po", bufs=3, space="PSUM") as pop,
    ):
        wp_sb = wpp.tile([P, MT, C], F32R)
        nc.sync.dma_start(
            out=wp_sb, in_=wp.bitcast(F32R).rearrange("(cp p) c -> p cp c", p=P)
        )
        for m in range(C // P):
            for s in range(TS):
                po = pop.tile([P, NS], F32, tag="po")
                for cp in range(MT):
                    nc.tensor.matmul(
                        po,
                        wp_sb[:, cp, m * P : (m + 1) * P],
                        attnT[:, cp, s * NS : (s + 1) * NS],
                        start=(cp == 0),
                        stop=(cp == MT - 1),
                    )
                po_sb = posp.tile([P, NS], F32, tag="po_sb")
                nc.vector.tensor_scalar_add(po_sb, po, bias_sb[:, m : m + 1])
                nc.sync.dma_start(
                    out=outT[m * P : (m + 1) * P, s * NS : (s + 1) * NS], in_=po_sb
                )


def build(T=T_FULL, reps=1, phases=(1, 2, 3)):
    from contextlib import ExitStack

    nc = bacc.Bacc(
        "TRN2", target_bir_lowering=False, debug=False, num_devices=NCORES
    )
    aps = {
        "x": nc.dram_tensor("x", [T, C], F32, kind="ExternalInput").ap(),
        "wr": nc.dram_tensor("wr", [C, CH], F32, kind="ExternalInput").ap(),
        "wc": nc.dram_tensor("wc", [C, CH], F32, kind="ExternalInput").ap(),
        "wv": nc.dram_tensor("wv", [C, CH], F32, kind="ExternalInput").ap(),
        "wp": nc.dram_tensor("wp", [CH, C], F32, kind="ExternalInput").ap(),
        "bias": nc.dram_tensor("bias", [P, C // P], F32, kind="ExternalInput").ap(),
        "outT": nc.dram_tensor("outT", [C, T], F32, kind="ExternalOutput").ap(),
    }
    with tile.TileContext(nc) as tc:
        with ExitStack() as ctx:
            if reps == 1:
                _emit(ctx, tc, aps, T, phases)
            else:
                with tc.For_i(0, reps, 1):
                    _emit(ctx, tc, aps, T, phases)
    nc.compile()
    return nc


def make_in_maps(x, Wk, Wq, Wv, Wproj, bproj):
    """Shard full inputs into 8 per-core input maps."""
    in_maps = []
    for c in range(NCORES):
        b, g = c // 2, c % 2
        cols = slice(CH * g, CH * (g + 1))
        in_maps.append(
            {
                "x": np.ascontiguousarray(np.asarray(x)[b], dtype=np.float32),
                "wr": np.ascontiguousarray(np.asarray(Wk)[:, cols], dtype=np.float32),
                "wc": np.ascontiguousarray(np.asarray(Wq)[:, cols], dtype=np.float32),
                "wv": np.ascontiguousarray(np.asarray(Wv)[:, cols], dtype=np.float32),
                "wp": np.ascontiguousarray(np.asarray(Wproj)[cols, :], dtype=np.float32),
                "bias": np.ascontiguousarray(
                    (0.5 * np.asarray(bproj)).reshape(C // P, P).T, dtype=np.float32
                ),
            }
        )
    return in_maps


_CACHE = {}


def kernel(x, Wk, Wq, Wv, Wproj, bproj):
    x = np.asarray(x, dtype=np.float32)
    if "nc" not in _CACHE:
        _CACHE["nc"] = build(T=x.shape[1])
    nc = _CACHE["nc"]
    in_maps = make_in_maps(x, Wk, Wq, Wv, Wproj, bproj)
    res = run_bass_kernel_spmd(nc, in_maps, list(range(NCORES)))
    out = np.empty((x.shape[0], x.shape[1], C), dtype=np.float32)
    for b in range(x.shape[0]):
        out[b] = (res.results[2 * b]["outT"] + res.results[2 * b + 1]["outT"]).T
    return out


# revision 17
# speedup vs baseline: 74.5599x; 1.1519x over previous
"""Trainium2 Bass kernel: causal multi-head attention (dense transformer block).

Reference semantics (note the quirk: scores = K @ Q^T, scaled by C**-0.5):
    k = x @ Wk ; q = x @ Wq ; v = x @ Wv          (per-head split, H=16, D=64)
    wei[i, j] = (k_i . q_j) * C**-0.5,  masked j <= i, softmax over j
    out = (wei @ v) @ Wproj + bproj

Sharding (8 cores): data-parallel over batch (4) x tensor-parallel over
head-halves (2).  Core c handles batch c//2, heads [8*(c%2), 8*(c%2)+8).
Each core returns the transposed partial projection output
outT = ((attn_heads @ Wproj_shard) + bproj/2)^T ; the host sums the two
partials per batch and transposes back.

Device kernel layout choices:
  - "row" operand Qr := x @ Wk_shard, "col" operand Kc := x @ Wq_shard.
  - Qr^T / Kc^T are kept head-transposed in SBUF as [128 (= 2 heads x 64d), T].
  - scores are computed transposed: S^T[j, i] = Qr_i . Kc_j via
    lhsT = KcT (stationary), rhs = QrT (moving); two heads packed in the PE
    array via row groups (K=64 each, tile_position (0,0)/(64,0)).
  - softmax: exp on ACT (scale fused), causal mask via gpsimd.affine_select.
    The denominator is folded into the AV matmul: V is augmented with a ones
    column (M=65), so PSUM row 64 of the AV output accumulates sum_j wei.
    Reciprocal = exp(-ln(d)) (exp and ln share one ACT table set), broadcast
    across partitions via a DRAM-bounce DMA with a partition-broadcast AP.
  - AV: lhsT = V tiles (natural layout), rhs = exp'd S^T -> attn^T, which is
    exactly the layout the output projection wants as its moving operand.
  - all matmuls run as float32r (fp32 data, FP22 multiply): full PE rate for
    moving dim >= 256.
"""

import numpy as np

import concourse.bass as bass  # noqa: F401
import concourse.tile as tile
from concourse import bacc, mybir
from concourse.bass_utils import run_bass_kernel_spmd
from concourse.masks import make_identity

F32 = mybir.dt.float32
F32R = mybir.dt.float32r

B, T_FULL, C = 4, 2048, 1024
H, NCORES, P, NS, D = 16, 8, 128, 512, 64
CH = C // 2            # per-core channels (8 heads)
HPC = CH // D          # heads per core
SCALE = 1.0 / float(np.sqrt(C))


def _emit(ctx, tc, aps, T, phases=(1, 2, 3)):
    nc = tc.nc
    x, wr, wc, wv, wp, bias_d, outT = (
        aps[k] for k in ("x", "wr", "wc", "wv", "wp", "bias", "outT")
    )
    KT = C // P           # 8 k-tiles of the model dim
    TT = T // P           # t-tiles
    TS = T // NS          # i-spans
    MT = CH // P          # 4 head-pair tiles
    CHUNK = NS
    NCH = T // CHUNK
    Exp = mybir.ActivationFunctionType.Exp
    Ln = mybir.ActivationFunctionType.Ln
    Copy = mybir.ActivationFunctionType.Copy

    consts = ctx.enter_context(tc.tile_pool(name="consts", bufs=1))
    ident = consts.tile([P, P], F32)
    make_identity(nc, ident)
    onesf = consts.tile([P, P], F32)
    nc.vector.memset(onesf, 1.0)
    bias_sb = consts.tile([P, C // P], F32)
    nc.sync.dma_start(out=bias_sb, in_=bias_d)

    acts = ctx.enter_context(tc.tile_pool(name="acts", bufs=1))
    qrT = acts.tile([P, MT, T], F32R)      # row operand, head-transposed
    kcT = acts.tile([P, MT, T], F32R)      # col operand, head-transposed
    v_sb = acts.tile([P, TT, HPC, D + 1], F32R)  # V + ones column per head
    # ones column (col 64 of every head slot), written once
    nc.vector.tensor_copy(
        v_sb[:, :, :, D : D + 1],
        onesf[:, 0 : TT * HPC].rearrange("p (a b c) -> p a b c", a=TT, b=HPC, c=1),
    )

    # ---------------- phase 1: projections ----------------
    if 1 not in phases:
        return
    with (
        tc.tile_pool(name="wqkv", bufs=1) as wpool,
        tc.tile_pool(name="xn", bufs=2) as xnp,
        tc.tile_pool(name="xt", bufs=2) as xtp,
        tc.tile_pool(name="pt", bufs=2, space="PSUM") as ptp,
        tc.tile_pool(name="pqv", bufs=3, space="PSUM") as pqp,
    ):
        wr_sb = wpool.tile([P, KT, CH], F32R)
        wc_sb = wpool.tile([P, KT, CH], F32R)
        wv_sb = wpool.tile([P, KT, CH], F32R)
        for w_dram, w_sb in ((wr, wr_sb), (wc, wc_sb), (wv, wv_sb)):
            nc.gpsimd.dma_start(
                out=w_sb,
                in_=w_dram.bitcast(F32R).rearrange("(kt p) ch -> p kt ch", p=P),
            )
        for chk in range(NCH):
            t0 = chk * CHUNK
            ntl = CHUNK // P
            xt = xtp.tile([P, KT, CHUNK], F32R)
            for tl in range(ntl):
                xn = xnp.tile([P, C], F32)
                nc.sync.dma_start(out=xn, in_=x[t0 + tl * P : t0 + (tl + 1) * P, :])
                for kt in range(KT):
                    pt = ptp.tile([P, P], F32)
                    nc.tensor.transpose(pt, xn[:, kt * P : (kt + 1) * P], ident)
                    nc.vector.tensor_copy(xt[:, kt, tl * P : (tl + 1) * P], pt)
            for w_sb, dst in ((wr_sb, qrT), (wc_sb, kcT)):
                for m in range(MT):
                    pq = pqp.tile([P, CHUNK], F32, tag="pqv")
                    for kt in range(KT):
                        nc.tensor.matmul(
                            pq,
                            w_sb[:, kt, m * P : (m + 1) * P],
                            xt[:, kt, :],
                            start=(kt == 0),
                            stop=(kt == KT - 1),
                        )
                    nc.scalar.activation(dst[:, m, t0 : t0 + CHUNK], pq, Copy)
            for tl in range(ntl):
                pv = pqp.tile([P, CH], F32, tag="pqv")
                for kt in range(KT):
                    nc.tensor.matmul(
                        pv,
                        xt[:, kt, tl * P : (tl + 1) * P],
                        wv_sb[:, kt, :],
                        start=(kt == 0),
                        stop=(kt == KT - 1),
                    )
                nc.scalar.activation(
                    v_sb[:, chk * ntl + tl, :, 0:D],
                    pv.rearrange("p (h d) -> p h d", h=HPC),
                    Copy,
                )

    # ------- phases 2+3: attention with interleaved output projection -------
    attn_pool = ctx.enter_context(tc.tile_pool(name="attn", bufs=1))
    attnT = attn_pool.tile([P, MT, T], F32R)
    if 2 not in phases:
        return
    with (
        tc.tile_pool(name="wei", bufs=4) as weip,
        tc.tile_pool(name="lnr", bufs=2) as lnrp,
        tc.tile_pool(name="bcs", bufs=2) as bcsp,
        tc.tile_pool(name="rdram", bufs=2, space="DRAM") as rdp,
        tc.tile_pool(name="psc", bufs=2, space="PSUM") as pscp,
        tc.tile_pool(name="pava", bufs=2, space="PSUM") as pavap,
        tc.tile_pool(name="pavb", bufs=2, space="PSUM") as pavbp,
    ):
        units = [
            (p, s, jt)
            for p in range(MT)
            for s in range(TS)
            for jt in range((s + 1) * (NS // P))
        ]
        DEPTH = 2
        pend = {}
        pavs = {}

        def emit_front(u):
            p, s, jt = u
            i0 = s * NS
            off = max(0, jt * P - i0)
            psc = pscp.tile([P, 2, NS], F32, tag="psc", name="psc")
            for h in range(2):
                hp = slice(64 * h, 64 * h + 64)
                nc.tensor.matmul(
                    psc[:, h, off:NS],
                    kcT[hp, p, jt * P : (jt + 1) * P],
                    qrT[hp, p, i0 + off : i0 + NS],
                    start=True,
                    stop=True,
                    tile_position=(64 * h, 0),
                )
            wei = weip.tile([P, 2, NS], F32R, tag="wei", name="wei")
            nc.scalar.activation(
                wei[:, :, off:NS], psc[:, :, off:NS], Exp, scale=SCALE
            )
            if jt * P >= i0:  # tile containing the diagonal
                nc.gpsimd.affine_select(
                    out=wei[:, :, off : off + P],
                    in_=wei[:, :, off : off + P],
                    pattern=[[0, 2], [1, P]],
                    base=0,
                    channel_multiplier=-1,
                    compare_op=mybir.AluOpType.is_ge,
                    fill=0.0,
                )
            pend[u] = wei

        def emit_back(u):
            p, s, jt = u
            i0 = s * NS
            jmax = (s + 1) * (NS // P)
            off = max(0, jt * P - i0)
            first, last = jt == 0, jt == jmax - 1
            wei = pend.pop(u)
            if first:
                pavs[(p, s)] = [
                    pavap.tile([D + 1, NS], F32, tag="pava", name="pava"),
                    pavbp.tile([D + 1, NS], F32, tag="pavb", name="pavb"),
                ]
            pav = pavs[(p, s)]
            for h in range(2):
                nc.tensor.matmul(
                    pav[h][:, off:NS],
                    v_sb[:, jt, 2 * p + h, :],
                    wei[:, h, off:NS],
                    start=first,
                    stop=last,
                )
            if not last:
                return
            # normalize: attnT[:, p, span] = pav[0:64] * 1/pav[64]
            pav = pavs.pop((p, s))
            lr = lnrp.tile([P, 2, NS], F32, tag="lr", name="lr")
            rab = lnrp.tile([P, 2, NS], F32, tag="rab", name="rab")
            for h in range(2):
                nc.scalar.activation(lr[D : D + 1, h, :], pav[h][D : D + 1, :], Ln)
                nc.scalar.activation(
                    rab[D : D + 1, h, :], lr[D : D + 1, h, :], Exp, scale=-1.0
                )
            rd = rdp.tile([1, 2, NS], F32, tag="rd", name="rd")
            nc.gpsimd.dma_start(out=rd[0], in_=rab[D : D + 1, :, :])
            bcs = bcsp.tile([D, 2, NS], F32, tag="bcs", name="bcs")
            nc.gpsimd.dma_start(out=bcs, in_=rd.to_broadcast([D, 2, NS]))
            for h in range(2):
                nc.vector.tensor_mul(
                    attnT[64 * h : 64 * h + 64, p, s * NS : (s + 1) * NS],
                    pav[h][0:D, :],
                    bcs[:, h, :],
                )

        for idx in range(len(units) + DEPTH):
            if idx < len(units):
                emit_front(units[idx])
            if idx >= DEPTH:
                emit_back(units[idx - DEPTH])

    # ---------------- phase 3: output projection ----------------
    if 3 not in phases:
        return
    with (
        tc.tile_pool(name="wpp", bufs=1) as wpp,
        tc.tile_pool(name="po_sb", bufs=3) as posp,
        tc.tile_pool(name="po", bufs=3, space="PSUM") as pop,
    ):
        wp_sb = wpp.tile([P, MT, C], F32R)
        nc.sync.dma_start(
            out=wp_sb, in_=wp.bitcast(F32R).rearrange("(cp p) c -> p cp c", p=P)
        )
        for m in range(C // P):
            for s in range(TS):
                po = pop.tile([P, NS], F32, tag="po")
                for cp in range(MT):
                    nc.tensor.matmul(
                        po,
                        wp_sb[:, cp, m * P : (m + 1) * P],
                        attnT[:, cp, s * NS : (s + 1) * NS],
                        start=(cp == 0),
                        stop=(cp == MT - 1),
                    )
                po_sb = posp.tile([P, NS], F32, tag="po_sb")
                nc.vector.tensor_scalar_add(po_sb, po, bias_sb[:, m : m + 1])
                nc.sync.dma_start(
                    out=outT[m * P : (m + 1) * P, s * NS : (s + 1) * NS], in_=po_sb
                )


def build(T=T_FULL, reps=1, phases=(1, 2, 3)):
    from contextlib import ExitStack

    nc = bacc.Bacc(
        "TRN2", target_bir_lowering=False, debug=False, num_devices=NCORES
    )
    aps = {
        "x": nc.dram_tensor("x", [T, C], F32, kind="ExternalInput").ap(),
        "wr": nc.dram_tensor("wr", [C, CH], F32, kind="ExternalInput").ap(),
        "wc": nc.dram_tensor("wc", [C, CH], F32, kind="ExternalInput").ap(),
        "wv": nc.dram_tensor("wv", [C, CH], F32, kind="ExternalInput").ap(),
        "wp": nc.dram_tensor("wp", [CH, C], F32, kind="ExternalInput").ap(),
        "bias": nc.dram_tensor("bias", [P, C // P], F32, kind="ExternalInput").ap(),
        "outT": nc.dram_tensor("outT", [C, T], F32, kind="ExternalOutput").ap(),
    }
    with tile.TileContext(nc) as tc:
        with ExitStack() as ctx:
            if reps == 1:
                _emit(ctx, tc, aps, T, phases)
            else:
                with tc.For_i(0, reps, 1):
                    _emit(ctx, tc, aps, T, phases)
    nc.compile()
    return nc


def make_in_maps(x, Wk, Wq, Wv, Wproj, bproj):
    """Shard full inputs into 8 per-core input maps."""
    in_maps = []
    for c in range(NCORES):
        b, g = c // 2, c % 2
        cols = slice(CH * g, CH * (g + 1))
        in_maps.append(
            {
                "x": np.ascontiguousarray(np.asarray(x)[b], dtype=np.float32),
                "wr": np.ascontiguousarray(np.asarray(Wk)[:, cols], dtype=np.float32),
                "wc": np.ascontiguousarray(np.asarray(Wq)[:, cols], dtype=np.float32),
                "wv": np.ascontiguousarray(np.asarray(Wv)[:, cols], dtype=np.float32),
                "wp": np.ascontiguousarray(np.asarray(Wproj)[cols, :], dtype=np.float32),
                "bias": np.ascontiguousarray(
                    (0.5 * np.asarray(bproj)).reshape(C // P, P).T, dtype=np.float32
                ),
            }
        )
    return in_maps


_CACHE = {}


def kernel(x, Wk, Wq, Wv, Wproj, bproj):
    x = np.asarray(x, dtype=np.float32)
    if "nc" not in _CACHE:
        _CACHE["nc"] = build(T=x.shape[1])
    nc = _CACHE["nc"]
    in_maps = make_in_maps(x, Wk, Wq, Wv, Wproj, bproj)
    res = run_bass_kernel_spmd(nc, in_maps, list(range(NCORES)))
    out = np.empty((x.shape[0], x.shape[1], C), dtype=np.float32)
    for b in range(x.shape[0]):
        out[b] = (res.results[2 * b]["outT"] + res.results[2 * b + 1]["outT"]).T
    return out
